# revision 1
# baseline (speedup 1.0000x reference)
#!/usr/bin/env python
"""Trainium2 Bass kernel for nn_Continuous_Tucker (SIREN x3 + Tucker core).

Data-parallel over the batch across 8 NeuronCores. Each core computes, for
its 8192-element batch slice:
  U/V/W = siren(x_i) for the three 1->512->512->32 nets with
          sine_layer(x) = sin(sin(4*(x @ W.T + b)))
  out[b] = sum_{r,s,t} U[b,r] V[b,s] W[b,t] C[r,s,t]

Key device-side design:
- Everything runs features-on-partitions; batch along the free dim.
- ACT's Sin spline is only valid on ~[-3.555, 3.555]. Layer-1 args reach
  +-8, so layer 1 works in "turns": f = w'*x_c + c'' with w' = 2*w1/pi,
  x_c = x - 0.5, and the per-feature phase c'' host-folded into [-1/4, 1/4]
  (mod 1, with a per-feature half-turn shift whose sign flip is absorbed
  into the next layer's weight columns). Then |2*pi*f| <= ~3.57 and
  sin(2*pi*f) = sin(4*(w1*x + b1)) exactly. One DVE op per chunk.
- Layer-2 args are bounded by 4*(0.8415*max_row_sum|w2| + max|b2|) < 3.55
  for these weight fills (asserted on host).
- Matmuls run in float32r (1 cycle/row vs 4 for fp32; ~1.6e-4 rel err).
- Tucker contraction: K2[(s,t), b] = V[s,b]*W[t,b] built via pattern
  replication matmuls + DVE products, then T2 = C3.T @ K2 accumulated in
  PSUM over 8 K-chunks, final dot with U via DVE + ones-matmul reduce.
"""
import sys

for _p in ("/opt/trn_rl_repo", "/root/.axon_site/_ro/trn_rl_repo"):
    if _p not in sys.path:
        sys.path.insert(0, _p)

import numpy as np

import concourse.bass as bass
import concourse.mybir as mybir
import concourse.tile as tile
from concourse import bacc
from concourse.bass_utils import run_bass_kernel_spmd

f32 = mybir.dt.float32
f32r = mybir.dt.float32r
AF = mybir.ActivationFunctionType
OP = mybir.AluOpType

N_CORES = 8
B = 65536
B_CORE = B // N_CORES
SUPER = 1024
NSUP = B_CORE // SUPER
NS = SUPER // 512  # 512-wide subtiles per super-tile
MID = 512
R = 32
OMEGA = 4.0
TWO_PI = float(2.0 * np.pi)

_CACHE = {}
import os
REPEAT = int(os.environ.get("KREPEAT", "1"))
KLOOP = int(os.environ.get("KLOOP", "0"))
MMDT = {"f32r": mybir.dt.float32r, "fp16": mybir.dt.float16}[
    os.environ.get("MMDT", "f32r")
]
NOXBC = os.environ.get("NOXBC", "0") == "1"  # timing probe: hoist x bcast
SINHALF = os.environ.get("SINHALF", "0") == "1"  # timing probe: 1 sin per layer


def _build_body(nc, tc, d, out):
    with (
        tc.tile_pool(name="const", bufs=1) as const,
        tc.tile_pool(name="acts", bufs=1) as acts,
        tc.tile_pool(name="work", bufs=2) as work,
        tc.tile_pool(name="ps_l2", bufs=3, space="PSUM") as ps_l2,
        tc.tile_pool(name="ps_med", bufs=2, space="PSUM") as ps_med,
        tc.tile_pool(name="ps_sm", bufs=1, space="PSUM") as ps_sm,
    ):
        _body_inner(
            nc, tc, d, out, const, acts, work, ps_l2, ps_med, ps_sm
        )


def _body_inner(nc, tc, d, out, const, acts, work, ps_l2, ps_med, ps_sm):
    # ---- constants into SBUF
    w2sb = [
        [const.tile([128, MID], MMDT, name=f"w2sb_{n}_{k}") for k in range(4)]
        for n in range(3)
    ]
    w3sb = [
        [const.tile([128, R], MMDT, name=f"w3sb_{n}_{k}") for k in range(4)]
        for n in range(3)
    ]
    b2sb, b3sb, wpsb, c2sb = [], [], [], []
    for n in range(3):
        for k in range(4):
            nc.gpsimd.dma_start(out=w2sb[n][k], in_=d["w2t"].ap()[n, k])
            nc.gpsimd.dma_start(out=w3sb[n][k], in_=d["w3t"].ap()[n, k])
        t = const.tile([128, 4], f32, name=f"b2sb_{n}")
        nc.sync.dma_start(out=t, in_=d["b2c"].ap()[n])
        b2sb.append(t)
        t = const.tile([R, 1], f32, name=f"b3sb_{n}")
        nc.sync.dma_start(out=t, in_=d["b3c"].ap()[n])
        b3sb.append(t)
        t = const.tile([128, 4], f32, name=f"wpsb_{n}")
        nc.sync.dma_start(out=t, in_=d["wpc"].ap()[n])
        wpsb.append(t)
        t = const.tile([128, 4], f32, name=f"c2sb_{n}")
        nc.sync.dma_start(out=t, in_=d["c2c"].ap()[n])
        c2sb.append(t)
    pwsb = const.tile([R, 128], MMDT, name="pwsb")
    nc.gpsimd.dma_start(out=pwsb, in_=d["pw"].ap())
    pvsb = [const.tile([R, 128], MMDT, name=f"pvsb_{c}") for c in range(8)]
    c3sb = [const.tile([128, R], MMDT, name=f"c3sb_{c}") for c in range(8)]
    for c in range(8):
        nc.gpsimd.dma_start(out=pvsb[c], in_=d["pv"].ap()[c])
        nc.gpsimd.dma_start(out=c3sb[c], in_=d["c3"].ap()[c])
    onesb = const.tile([R, 1], MMDT, name="onesb")
    nc.gpsimd.dma_start(out=onesb, in_=d["ones_r"].ap())

    out2d = out.ap().rearrange("(a b) -> a b", a=NSUP * NS)
    x_ap = d["xc"].ap()

    import contextlib

    loop_cm = tc.For_i(0, KLOOP, 1) if KLOOP > 0 else contextlib.nullcontext()
    with loop_cm:
        _main_loops(
            nc, d, out2d, x_ap, const, acts, work, ps_l2, ps_med, ps_sm,
            w2sb, w3sb, b2sb, b3sb, wpsb, c2sb, pwsb, pvsb, c3sb, onesb,
        )


def _main_loops(
    nc, d, out2d, x_ap, const, acts, work, ps_l2, ps_med, ps_sm,
    w2sb, w3sb, b2sb, b3sb, wpsb, c2sb, pwsb, pvsb, c3sb, onesb,
):
    xbc_fixed = []
    if NOXBC:
        for n in range(3):
            t = const.tile([128, SUPER], f32, name=f"xbcf{n}")
            nc.gpsimd.dma_start(
                out=t,
                in_=bass.AP(
                    tensor=x_ap.tensor,
                    offset=n * B_CORE,
                    ap=[[0, 128], [1, SUPER]],
                ),
            )
            xbc_fixed.append(t)
    for st in [s for _ in range(REPEAT) for s in range(NSUP)]:
        uvw = []
        for n in range(3):
            # ---- x broadcast + layer-1 affine + layer-1 sines
            if NOXBC:
                xbc = xbc_fixed[n]
            else:
                xbc = work.tile([128, SUPER], f32, name="xbc", tag="xbc", bufs=2)
                srcap = bass.AP(
                    tensor=x_ap.tensor,
                    offset=n * B_CORE + st * SUPER,
                    ap=[[0, 128], [1, SUPER]],
                )
                nc.gpsimd.dma_start(out=xbc, in_=srcap)
            ftile = acts.tile(
                [128, 4, SUPER], f32, name="ftile", tag="f", bufs=2
            )
            for m in range(4):
                nc.vector.tensor_scalar(
                    ftile[:, m, :],
                    xbc,
                    wpsb[n][:, m : m + 1],
                    c2sb[n][:, m : m + 1],
                    OP.mult,
                    OP.add,
                )
            h1 = acts.tile(
                [128, 4, SUPER], MMDT, name="h1", tag=f"h1{n}", bufs=1
            )
            if SINHALF:
                nc.scalar.activation(h1, ftile, AF.Sin, scale=TWO_PI)
            else:
                nc.scalar.activation(ftile, ftile, AF.Sin, scale=TWO_PI)
                nc.scalar.activation(h1, ftile, AF.Sin)

            # ---- layer 2 matmuls; sines per 2-m-chunk half so ACT starts early
            h2 = acts.tile(
                [128, 4, SUPER], MMDT, name="h2", tag=f"h2{n}", bufs=1
            )
            for m in range(4):
                pts = [
                    ps_l2.tile([128, 512], f32, name=f"l2ps{ns}", tag="l2")
                    for ns in range(NS)
                ]
                for k in range(4):
                    for ns in range(NS):
                        nc.tensor.matmul(
                            pts[ns],
                            lhsT=w2sb[n][k][:, m * 128 : (m + 1) * 128],
                            rhs=h1[:, k, ns * 512 : (ns + 1) * 512],
                            start=(k == 0),
                            stop=(k == 3),
                        )
                for ns in range(NS):
                    nc.vector.tensor_scalar_add(
                        h2[:, m, ns * 512 : (ns + 1) * 512],
                        pts[ns],
                        b2sb[n][:, m : m + 1],
                    )
                if m % 2 == 1:
                    half = h2[:, m - 1 : m + 1, :]
                    nc.scalar.activation(half, half, AF.Sin, scale=OMEGA)
                    if not SINHALF:
                        nc.scalar.activation(half, half, AF.Sin)

            # ---- layer 3
            uv = acts.tile(
                [R, SUPER],
                f32 if n == 0 else MMDT,
                name=f"uvw{n}",
                tag=f"uvw{n}",
                bufs=1,
            )
            for ns in range(NS):
                pt3 = ps_sm.tile([R, 512], f32, name="l3ps", tag="l3orow")
                for k in range(4):
                    nc.tensor.matmul(
                        pt3,
                        lhsT=w3sb[n][k],
                        rhs=h2[:, k, ns * 512 : (ns + 1) * 512],
                        start=(k == 0),
                        stop=(k == 3),
                    )
                nc.vector.tensor_scalar_add(
                    uv[:, ns * 512 : (ns + 1) * 512], pt3, b3sb[n]
                )
            uvw.append(uv)

        # ---- Tucker contraction (ns halves interleaved for overlap)
        U, V, W = uvw
        wrep = work.tile([128, SUPER], f32, name="wrep", tag="wrep", bufs=2)
        for ns in range(NS):
            nsl = slice(ns * 512, (ns + 1) * 512)
            ptw = ps_med.tile([128, 512], f32, name="wrep_ps", tag="med")
            nc.tensor.matmul(ptw, lhsT=pwsb, rhs=W[:, nsl], start=True, stop=True)
            nc.vector.tensor_copy(wrep[:, nsl], ptw)
        t2 = ps_sm.tile([R, SUPER], f32, name="t2ps", tag="t2")
        for c in range(8):
            for ns in range(NS):
                nsl = slice(ns * 512, (ns + 1) * 512)
                ptv = ps_med.tile([128, 512], f32, name="vrep_ps", tag="med")
                nc.tensor.matmul(
                    ptv, lhsT=pvsb[c], rhs=V[:, nsl], start=True, stop=True
                )
                k2 = work.tile([128, 512], MMDT, name="k2", tag="k2", bufs=4)
                nc.vector.tensor_mul(k2, ptv, wrep[:, nsl])
                nc.tensor.matmul(
                    t2[:, nsl], lhsT=c3sb[c], rhs=k2,
                    start=(c == 0), stop=(c == 7),
                )
        for ns in range(NS):
            nsl = slice(ns * 512, (ns + 1) * 512)
            m3 = work.tile([R, 512], MMDT, name="m3", tag="m3", bufs=2)
            nc.vector.tensor_mul(m3, t2[:, nsl], U[:, nsl])
            pto = ps_sm.tile([1, 512], f32, name="orow_ps", tag="l3orow")
            nc.tensor.matmul(pto, lhsT=onesb, rhs=m3, start=True, stop=True)
            orow = work.tile([1, 512], f32, name="orow", tag="orow", bufs=2)
            nc.vector.tensor_copy(orow, pto)
            nc.sync.dma_start(
                out=out2d[st * NS + ns : st * NS + ns + 1, :], in_=orow
            )


def _build_nc():
    nc = bacc.Bacc(
        "TRN2", target_bir_lowering=False, debug=False, num_devices=N_CORES
    )
    d = {}
    for name, shape in (
        ("xc", (3, B_CORE)),
        ("w2t", (3, 4, 128, MID)),
        ("b2c", (3, 128, 4)),
        ("w3t", (3, 4, 128, R)),
        ("b3c", (3, R, 1)),
        ("wpc", (3, 128, 4)),
        ("c2c", (3, 128, 4)),
        ("pw", (R, 128)),
        ("pv", (8, R, 128)),
        ("c3", (8, 128, R)),
        ("ones_r", (R, 1)),
    ):
        d[name] = nc.dram_tensor(name, shape, f32, kind="ExternalInput")
    out = nc.dram_tensor("out", (B_CORE,), f32, kind="ExternalOutput")
    with tile.TileContext(nc) as tc:
        _build_body(nc, tc, d, out)
    nc.compile()
    return nc


def prep_weights(inputs):
    """Host-side packing of all weight-derived device inputs (core-independent)."""
    w = {}
    ww = {k: np.asarray(v, np.float32) for k, v in inputs.items()}
    w2t = np.empty((3, 4, 128, MID), np.float32)
    b2c = np.empty((3, 128, 4), np.float32)
    w3t = np.empty((3, 4, 128, R), np.float32)
    b3c = np.empty((3, R, 1), np.float32)
    wpc = np.empty((3, 128, 4), np.float32)
    c2c = np.empty((3, 128, 4), np.float32)
    for n, pfx in enumerate(("U", "V", "W")):
        w1 = ww[pfx + "w1"][:, 0]  # (512,)
        b1 = ww[pfx + "b1"]
        w2 = ww[pfx + "w2"]
        b2 = ww[pfx + "b2"]
        w3 = ww[pfx + "w3"]
        b3 = ww[pfx + "b3"]
        # layer-2 arg domain check (ACT sin valid |arg| <= ~3.555)
        bound = OMEGA * (
            np.sin(1.0) * np.abs(w2).sum(axis=1).max() + np.abs(b2).max()
        )
        assert bound < 3.55, f"layer-2 sin arg bound {bound} exceeds ACT domain"
        # layer-1 turns: f = w'*(x-0.5) + c'' ; sign flips into w2 columns
        wp = np.float64(2.0 / np.pi) * w1.astype(np.float64)  # 4*w1/(2pi)
        c0 = np.float64(2.0 / np.pi) * b1.astype(np.float64) + 0.5 * wp
        c1 = c0 - np.round(c0)
        flip = np.abs(c1) > 0.25
        c2 = np.where(flip, c1 - 0.5 * np.sign(c1), c1)
        F = np.where(flip, -1.0, 1.0)
        w2_eff = (w2.astype(np.float64) * F[None, :]).astype(np.float32)
        w2t[n] = w2_eff.T.reshape(4, 128, MID)
        b2c[n] = b2.reshape(4, 128).T
        w3t[n] = w3.T.reshape(4, 128, R).astype(np.float32)
        b3c[n] = b3.reshape(R, 1)
        wpc[n] = wp.astype(np.float32).reshape(4, 128).T
        c2c[n] = c2.astype(np.float32).reshape(4, 128).T
    w["w2t"], w["b2c"], w["w3t"], w["b3c"] = w2t, b2c, w3t, b3c
    w["wpc"], w["c2c"] = wpc, c2c
    # Tucker patterns and matricized core
    q = np.arange(128)
    pw = (q[None, :] % R == np.arange(R)[:, None]).astype(np.float32)
    pv = np.zeros((8, R, 128), np.float32)
    c3 = np.empty((8, 128, R), np.float32)
    C = ww["core"].reshape(R, R, R)
    for c in range(8):
        s = 4 * c + q // 32
        pv[c][s, q] = 1.0
        c3[c] = C[:, s, q % 32].T
    w["pw"], w["pv"], w["c3"] = pw, pv, c3
    w["ones_r"] = np.ones((R, 1), np.float32)
    return w


def make_in_maps(inputs):
    w = prep_weights(inputs)
    x = np.asarray(inputs["train_ind_batch"], np.float32)
    in_maps = []
    for c in range(N_CORES):
        sl = x[c * B_CORE : (c + 1) * B_CORE]
        m = dict(w)
        m["xc"] = np.ascontiguousarray(sl.T) - 0.5
        in_maps.append(m)
    return in_maps


def get_nc():
    if "nc" not in _CACHE:
        _CACHE["nc"] = _build_nc()
    return _CACHE["nc"]


def kernel(**inputs) -> np.ndarray:
    nc = get_nc()
    in_maps = make_in_maps(inputs)
    res = run_bass_kernel_spmd(nc, in_maps, core_ids=list(range(N_CORES)))
    return np.concatenate(
        [res.results[c]["out"] for c in range(N_CORES)]
    ).astype(np.float32)


if __name__ == "__main__":
    rng = np.random.default_rng(0)
    # quick self-exercise with random data
    demo = {"train_ind_batch": rng.uniform(0, 1, (B, 3)).astype(np.float32)}
    for pfx in ("U", "V", "W"):
        demo[pfx + "w1"] = rng.uniform(-1, 1, (MID, 1)).astype(np.float32)
        demo[pfx + "b1"] = rng.uniform(-1, 1, MID).astype(np.float32)
        demo[pfx + "w2"] = rng.uniform(-1 / MID, 1 / MID, (MID, MID)).astype(
            np.float32
        )
        demo[pfx + "b2"] = rng.uniform(
            -1 / np.sqrt(MID), 1 / np.sqrt(MID), MID
        ).astype(np.float32)
        demo[pfx + "w3"] = rng.uniform(
            -1 / np.sqrt(MID), 1 / np.sqrt(MID), (R, MID)
        ).astype(np.float32)
        demo[pfx + "b3"] = rng.uniform(
            -1 / np.sqrt(MID), 1 / np.sqrt(MID), R
        ).astype(np.float32)
    demo["core"] = rng.standard_normal(R * R * R).astype(np.float32)
    out = kernel(**demo)
    print("out", out.shape, out[:4])



# revision 21
# speedup vs baseline: 1.1081x; 1.1081x over previous
#!/usr/bin/env python
"""Trainium2 Bass kernel for nn_Continuous_Tucker (SIREN x3 + Tucker core).

Data-parallel over the batch across 8 NeuronCores (8192 elements each).

Algorithm (device side):
  Each SIREN net U/V/W maps a SCALAR coordinate to R^32 and is extremely
  smooth (|w2| ~ 1/512), so instead of evaluating the 512-wide MLP for
  every batch element, the kernel:
    1. evaluates each net at 128 uniform grid points g_p = p/127 on
       device (exact same math as the MLP, batch=128 -> negligible cost),
       folding the +b3 bias into the grid values;
    2. linearly interpolates per batch element via a "hat" weight matrix
       S[p,b] = relu(1 - |127*x_b - p|)  (exactly 2 nonzeros per column),
       so U^T = G_u^T S etc. are plain matmuls.  Linear interp on this
       grid is accurate to ~7e-5 relative (tolerance is 2e-2).
    3. contracts the Tucker core: K2[(s,t),b] = V[s,b] W[t,b] built from
       partition-replicated V,W (stride-0 DMA broadcast), then
       T2 = C3^T K2 accumulated over 8 chunks in PSUM, final dot with U.

  Engine mapping per 1024-element supertile: x broadcast via rank-1
  matmul (PE), hat build Abs on ACT + min on DVE (the hat sign is negated
  and absorbed into the negated grid tables so one DVE op suffices),
  interpolation + core contraction on PE in fp16 (1 cycle/row), V/W
  replication on the DMA fabric, K2 product on DVE (fp16 2x mode) with a
  few chunks optionally on GPSIMD.

  Layer-1 sines use ACT's Sin (valid |arg| <= ~3.555) via the "turns"
  reduction: f = w'*(g-0.5) + c'' with c'' host-folded into [-1/4,1/4]
  (sign flips absorbed into layer-2 weight columns), then
  sin(2*pi*f) = sin(4*(w1*g + b1)) exactly.  Layer-2 args are bounded by
  4*(sin(1)*max_row_sum|w2| + max|b2|) < 3.55 (asserted on host).
"""
import os
import sys

for _p in ("/opt/trn_rl_repo", "/root/.axon_site/_ro/trn_rl_repo"):
    if _p not in sys.path:
        sys.path.insert(0, _p)

import numpy as np

import concourse.bass as bass
import concourse.mybir as mybir
import concourse.tile as tile
from concourse import bacc
from concourse.bass_utils import run_bass_kernel_spmd

f32 = mybir.dt.float32
f32r = mybir.dt.float32r
f16 = mybir.dt.float16
AF = mybir.ActivationFunctionType
OP = mybir.AluOpType

N_CORES = 8
B = 65536
B_CORE = B // N_CORES
SUPER = 1024
NSUP = B_CORE // SUPER
NS = SUPER // 512
MID = 512
R = 32
NG = 128          # grid points per net
NCELL = float(NG - 1)
OMEGA = 4.0
TWO_PI = float(2.0 * np.pi)

_CACHE = {}
KLOOP = int(os.environ.get("KLOOP", "0"))       # hardware-loop repeat (timing)
K2POOL = int(os.environ.get("K2POOL", "4"))     # k2 chunks on gpsimd
PSDMA = os.environ.get("PSDMA", "0") == "1"     # DMA outputs straight from PSUM


def _emit(nc, tc, d, out, P):
    """Emit one full kernel pass (const loads + grid eval + batch loop)."""
    const = P["const"]
    sbS = P["sbS"]
    work = P["work"]
    ps_zb = P["ps_zb"]
    ps_it = P["ps_it"]
    ps_t2 = P["ps_t2"]
    ps_o = P["ps_o"]

    # ---------------- constants into SBUF ----------------
    w2sb = [
        [const.tile([128, MID], f16, name=f"w2sb_{n}_{k}") for k in range(4)]
        for n in range(3)
    ]
    w3sb = [
        [const.tile([128, R], f16, name=f"w3sb_{n}_{k}") for k in range(4)]
        for n in range(3)
    ]
    wpsb, c2sb, b2sb4, nb3sb = [], [], [], []
    for n in range(3):
        for k in range(4):
            nc.gpsimd.dma_start(out=w2sb[n][k], in_=d["w2t16"].ap()[n, k])
            nc.gpsimd.dma_start(out=w3sb[n][k], in_=d["w3t16"].ap()[n, k])
        t = const.tile([128, 4], f32, name=f"wpsb_{n}")
        nc.sync.dma_start(out=t, in_=d["wpc"].ap()[n])
        wpsb.append(t)
        t = const.tile([128, 4], f32, name=f"c2sb_{n}")
        nc.sync.dma_start(out=t, in_=d["c2c"].ap()[n])
        c2sb.append(t)
        t = const.tile([128, 4], f32, name=f"b2sb4_{n}")
        nc.sync.dma_start(out=t, in_=d["b2s4"].ap()[n])
        b2sb4.append(t)
        t = const.tile([128, R], f32, name=f"nb3sb_{n}")
        nc.sync.dma_start(out=t, in_=d["nb3bc"].ap()[n])
        nb3sb.append(t)
    gbcsb = const.tile([128, NG], f32, name="gbcsb")
    nc.sync.dma_start(out=gbcsb, in_=d["gbc"].ap())
    npsb = const.tile([128, 1], f32, name="npsb")
    nc.sync.dma_start(out=npsb, in_=d["npvec"].ap())
    one128 = const.tile([1, 128], f32r, name="one128")
    nc.sync.dma_start(out=one128, in_=d["one128"].ap())
    ones32 = const.tile([R, 1], f16, name="ones32")
    nc.sync.dma_start(out=ones32, in_=d["ones32"].ap())
    c3sb = [const.tile([128, R], f16, name=f"c3sb_{c}") for c in range(8)]
    for c in range(8):
        nc.gpsimd.dma_start(out=c3sb[c], in_=d["c316"].ap()[c])

    # ---------------- grid eval: GT[n] = -(net_n(grid) + b3) ----------------
    GT = []
    with tc.tile_pool(name="ps_g", bufs=1, space="PSUM") as ps_g:
        for n in range(3):
            fg = work.tile([128, 4, NG], f32, name="fg", tag="fg", bufs=1)
            for m in range(4):
                nc.vector.tensor_scalar(
                    fg[:, m, :], gbcsb, wpsb[n][:, m : m + 1],
                    c2sb[n][:, m : m + 1], OP.mult, OP.add,
                )
            nc.scalar.activation(fg, fg, AF.Sin, scale=TWO_PI)
            h1g = work.tile([128, 4, NG], f16, name="h1g", tag="h1g", bufs=1)
            nc.scalar.activation(h1g, fg, AF.Sin)
            h2g = work.tile([128, 4, NG], f16, name="h2g", tag="h2g", bufs=1)
            for m in range(4):
                pg = ps_g.tile([128, NG], f32, name="pg", tag="pg")
                for k in range(4):
                    nc.tensor.matmul(
                        pg,
                        lhsT=w2sb[n][k][:, m * 128 : (m + 1) * 128],
                        rhs=h1g[:, k, :],
                        start=(k == 0),
                        stop=(k == 3),
                    )
                tg = work.tile([128, NG], f32, name="tg", tag="tg", bufs=2)
                nc.scalar.activation(
                    tg, pg, AF.Sin, bias=b2sb4[n][:, m : m + 1], scale=OMEGA
                )
                nc.scalar.activation(h2g[:, m, :], tg, AF.Sin)
            pgt = ps_g.tile([128, R], f32, name="pgt", tag="pg")
            for k in range(4):
                nc.tensor.matmul(
                    pgt, lhsT=h2g[:, k, :], rhs=w3sb[n][k],
                    start=(k == 0), stop=(k == 3),
                )
            gt = const.tile([128, R], f16, name=f"GT_{n}")
            nc.vector.tensor_sub(gt, nb3sb[n], pgt)  # -(G + b3)
            GT.append(gt)

    out2d = out.ap().rearrange("(a b) -> a b", a=NSUP)
    xr_ap = d["xr"].ap()

    # ---------------- batch supertile loop ----------------
    for st in range(NSUP):
        Usb = None
        Vsb = None
        Wsb = None
        xrow = work.tile([1, 3, SUPER], f32r, name="xrow", tag="xrow", bufs=2)
        nc.sync.dma_start(
            out=xrow,
            in_=xr_ap[:, st * SUPER : (st + 1) * SUPER].unsqueeze(0),
        )
        for n in range(3):
            # x broadcast to 128 partitions via rank-1 matmul
            zb = ps_zb.tile([128, NS, 512], f32, name="zb", tag="zb", bufs=1)
            for s2 in range(NS):
                nc.tensor.matmul(
                    zb[:, s2, :],
                    lhsT=one128,
                    rhs=xrow[:, n, s2 * 512 : (s2 + 1) * 512],
                    start=True, stop=True,
                )
            # t1 = |127*x - p|  (ACT), S = min(t1-1, 0) = -hat (DVE)
            t1 = work.tile([128, NS, 512], f16, name="t1", tag="t1", bufs=2)
            nc.scalar.activation(t1, zb, AF.Abs, bias=npsb, scale=NCELL)
            S = sbS.tile([128, NS, 512], f16, name="S", tag=f"S{n}", bufs=2)
            nc.vector.tensor_scalar(S, t1, 1.0, 0.0, OP.subtract, OP.min)
            # interpolate: psum = GT^T S  ->  copy to SBUF (U in f32 so the
            # final m3 product has uniform f32 inputs; V/W in f16 for k2)
            uvw = sbS.tile(
                [R, SUPER], f32 if n == 0 else f16,
                name=f"uvw{n}", tag=f"uvw{n}", bufs=2,
            )
            for s2 in range(NS):
                pv = ps_it.tile([R, 512], f32, name="pit", tag="it", bufs=1)
                nc.tensor.matmul(
                    pv, lhsT=GT[n], rhs=S[:, s2, :], start=True, stop=True
                )
                nc.scalar.activation(
                    uvw[:, s2 * 512 : (s2 + 1) * 512], pv, AF.Copy
                )
            if n == 0:
                Usb = uvw
            elif n == 1:
                Vsb = uvw
            else:
                Wsb = uvw

        # ---- V/W partition replication DMAs (leading AP dim must step)
        # wrep[p] = W[p % 32]: 4 plain row-block copies
        wrep = sbS.tile([128, SUPER], f16, name="wrep", tag="wrep", bufs=2)
        for j in range(4):
            nc.sync.dma_start(out=wrep[j * R : (j + 1) * R, :], in_=Wsb[:, :])
        # vrall[p, c] = V[4c + p//32]: per-chunk broadcast, src [4, 32, S]
        vrall = sbS.tile([128, 8, SUPER], f16, name="vrall", tag="vrall", bufs=2)
        for c in range(8):
            vsrc = (
                Vsb[4 * c : 4 * c + 4, :]
                .unsqueeze(1)
                .broadcast_to([4, R, SUPER])
            )
            nc.sync.dma_start(out=vrall[:, c, :], in_=vsrc)

        # ---- K2 product + core contraction
        t2 = ps_t2.tile([R, NS, 512], f32, name="t2", tag="t2", bufs=1)
        for c in range(8):
            k2 = work.tile([128, SUPER], f16, name="k2", tag="k2", bufs=3)
            eng = nc.gpsimd if c < K2POOL else nc.vector
            eng.tensor_mul(k2, vrall[:, c, :], wrep)
            for s2 in range(NS):
                nc.tensor.matmul(
                    t2[:, s2, :],
                    lhsT=c3sb[c],
                    rhs=k2[:, s2 * 512 : (s2 + 1) * 512],
                    start=(c == 0),
                    stop=(c == 7),
                )
        # ---- final dot with U and reduce over r
        po = ps_o.tile([1, NS, 512], f32, name="po", tag="po", bufs=1)
        for s2 in range(NS):
            m3 = work.tile([R, 512], f16, name="m3", tag="m3", bufs=2)
            nc.vector.tensor_mul(
                m3, t2[:, s2, :], Usb[:, s2 * 512 : (s2 + 1) * 512]
            )
            nc.tensor.matmul(
                po[:, s2, :], lhsT=ones32, rhs=m3, start=True, stop=True
            )
        if PSDMA:
            nc.sync.dma_start(out=out2d[st : st + 1, :], in_=po)
        else:
            orow = work.tile([1, NS, 512], f32, name="orow", tag="orow", bufs=2)
            nc.scalar.copy(orow, po)
            nc.sync.dma_start(out=out2d[st : st + 1, :], in_=orow)


def _build_body(nc, tc, d, out, kloop):
    import contextlib

    with (
        tc.tile_pool(name="const", bufs=1) as const,
        tc.tile_pool(name="sbS", bufs=1) as sbS,
        tc.tile_pool(name="work", bufs=1) as work,
        tc.tile_pool(name="ps_zb", bufs=1, space="PSUM") as ps_zb,
        tc.tile_pool(name="ps_it", bufs=1, space="PSUM") as ps_it,
        tc.tile_pool(name="ps_t2", bufs=1, space="PSUM") as ps_t2,
        tc.tile_pool(name="ps_o", bufs=1, space="PSUM") as ps_o,
    ):
        P = dict(const=const, sbS=sbS, work=work, ps_zb=ps_zb,
                 ps_it=ps_it, ps_t2=ps_t2, ps_o=ps_o)
        loop_cm = (
            tc.For_i(0, kloop, 1) if kloop > 0 else contextlib.nullcontext()
        )
        with loop_cm:
            _emit(nc, tc, d, out, P)


def build_nc(kloop=0):
    nc = bacc.Bacc(
        "TRN2", target_bir_lowering=False, debug=False, num_devices=N_CORES
    )
    d = {}
    specs = (
        ("xr", (3, B_CORE), f32r),
        ("wpc", (3, 128, 4), f32),
        ("c2c", (3, 128, 4), f32),
        ("b2s4", (3, 128, 4), f32),
        ("w2t16", (3, 4, 128, MID), f16),
        ("w3t16", (3, 4, 128, R), f16),
        ("nb3bc", (3, 128, R), f32),
        ("gbc", (128, NG), f32),
        ("npvec", (128, 1), f32),
        ("one128", (1, 128), f32r),
        ("ones32", (R, 1), f16),
        ("c316", (8, 128, R), f16),
    )
    for name, shape, dt in specs:
        d[name] = nc.dram_tensor(name, shape, dt, kind="ExternalInput")
    out = nc.dram_tensor("out", (B_CORE,), f32, kind="ExternalOutput")
    with tile.TileContext(nc) as tc:
        _build_body(nc, tc, d, out, kloop)
    nc.compile()
    return nc


def prep_weights(inputs):
    """Host-side packing of weight-derived device inputs (core-independent)."""
    w = {}
    ww = {k: np.asarray(v, np.float32) for k, v in inputs.items()}
    w2t16 = np.empty((3, 4, 128, MID), np.float16)
    w3t16 = np.empty((3, 4, 128, R), np.float16)
    wpc = np.empty((3, 128, 4), np.float32)
    c2c = np.empty((3, 128, 4), np.float32)
    b2s4 = np.empty((3, 128, 4), np.float32)
    nb3bc = np.empty((3, 128, R), np.float32)
    for n, pfx in enumerate(("U", "V", "W")):
        w1 = ww[pfx + "w1"][:, 0]
        b1 = ww[pfx + "b1"]
        w2 = ww[pfx + "w2"]
        b2 = ww[pfx + "b2"]
        w3 = ww[pfx + "w3"]
        b3 = ww[pfx + "b3"]
        # layer-2 arg domain check (ACT sin valid |arg| <= ~3.555)
        bound = OMEGA * (
            np.sin(1.0) * np.abs(w2).sum(axis=1).max() + np.abs(b2).max()
        )
        assert bound < 3.55, f"layer-2 sin arg bound {bound} exceeds ACT domain"
        # layer-1 turns: f = w'*(g-0.5) + c'' ; sign flips into w2 columns
        wp = np.float64(2.0 / np.pi) * w1.astype(np.float64)
        c0 = np.float64(2.0 / np.pi) * b1.astype(np.float64) + 0.5 * wp
        c1 = c0 - np.round(c0)
        flip = np.abs(c1) > 0.25
        c2f = np.where(flip, c1 - 0.5 * np.sign(c1), c1)
        F = np.where(flip, -1.0, 1.0)
        w2_eff = (w2.astype(np.float64) * F[None, :]).astype(np.float32)
        w2t16[n] = w2_eff.T.reshape(4, 128, MID).astype(np.float16)
        w3t16[n] = w3.T.reshape(4, 128, R).astype(np.float16)
        wpc[n] = wp.astype(np.float32).reshape(4, 128).T
        c2c[n] = c2f.astype(np.float32).reshape(4, 128).T
        b2s4[n] = (OMEGA * b2).reshape(4, 128).T
        nb3bc[n] = np.broadcast_to(-b3[None, :], (128, R))
    w["w2t16"], w["w3t16"] = w2t16, w3t16
    w["wpc"], w["c2c"], w["b2s4"], w["nb3bc"] = wpc, c2c, b2s4, nb3bc
    grid = np.arange(NG, dtype=np.float32) / np.float32(NCELL) - 0.5
    w["gbc"] = np.broadcast_to(grid[None, :], (128, NG)).copy()
    w["npvec"] = -np.arange(128, dtype=np.float32).reshape(128, 1)
    w["one128"] = np.ones((1, 128), np.float32)
    w["ones32"] = np.ones((R, 1), np.float16)
    c316 = np.empty((8, 128, R), np.float16)
    q = np.arange(128)
    C = ww["core"].reshape(R, R, R)
    for c in range(8):
        s = 4 * c + q // 32
        c316[c] = C[:, s, q % 32].T
    w["c316"] = c316
    return w


def make_in_maps(inputs):
    w = prep_weights(inputs)
    x = np.asarray(inputs["train_ind_batch"], np.float32)
    in_maps = []
    for c in range(N_CORES):
        sl = x[c * B_CORE : (c + 1) * B_CORE]
        m = dict(w)
        m["xr"] = np.ascontiguousarray(sl.T)
        in_maps.append(m)
    return in_maps


def get_nc():
    if "nc" not in _CACHE:
        _CACHE["nc"] = build_nc(KLOOP)
    return _CACHE["nc"]


def kernel(**inputs) -> np.ndarray:
    nc = get_nc()
    in_maps = make_in_maps(inputs)
    res = run_bass_kernel_spmd(nc, in_maps, core_ids=list(range(N_CORES)))
    return np.concatenate(
        [res.results[c]["out"] for c in range(N_CORES)]
    ).astype(np.float32)


if __name__ == "__main__":
    rng = np.random.default_rng(0)
    demo = {"train_ind_batch": rng.uniform(0, 1, (B, 3)).astype(np.float32)}
    for pfx in ("U", "V", "W"):
        demo[pfx + "w1"] = rng.uniform(-1, 1, (MID, 1)).astype(np.float32)
        demo[pfx + "b1"] = rng.uniform(-1, 1, MID).astype(np.float32)
        demo[pfx + "w2"] = rng.uniform(-1 / MID, 1 / MID, (MID, MID)).astype(
            np.float32
        )
        demo[pfx + "b2"] = rng.uniform(
            -1 / np.sqrt(MID), 1 / np.sqrt(MID), MID
        ).astype(np.float32)
        demo[pfx + "w3"] = rng.uniform(
            -1 / np.sqrt(MID), 1 / np.sqrt(MID), (R, MID)
        ).astype(np.float32)
        demo[pfx + "b3"] = rng.uniform(
            -1 / np.sqrt(MID), 1 / np.sqrt(MID), R
        ).astype(np.float32)
    demo["core"] = rng.standard_normal(R * R * R).astype(np.float32)
    out = kernel(**demo)
    print("out", out.shape, out[:4])


# revision 22
# speedup vs baseline: 195.3275x; 176.2755x over previous
#!/usr/bin/env python
"""Trainium2 Bass kernel for nn_Continuous_Tucker (SIREN x3 + Tucker core).

Data-parallel over the batch across 8 NeuronCores (8192 elements each).

Algorithm (device side):
  Each SIREN net U/V/W maps a SCALAR coordinate to R^32 and is extremely
  smooth (|w2| ~ 1/512), so instead of evaluating the 512-wide MLP for
  every batch element, the kernel:
    1. evaluates each net at 128 uniform grid points g_p = p/127 on
       device (exact same math as the MLP, batch=128 -> negligible cost),
       folding the +b3 bias into the grid values;
    2. linearly interpolates per batch element via a "hat" weight matrix
       S[p,b] = relu(1 - |127*x_b - p|)  (exactly 2 nonzeros per column),
       so U^T = G_u^T S etc. are plain matmuls.  Linear interp on this
       grid is accurate to ~7e-5 relative (tolerance is 2e-2).
    3. contracts the Tucker core: K2[(s,t),b] = V[s,b] W[t,b] built from
       partition-replicated V,W (stride-0 DMA broadcast), then
       T2 = C3^T K2 accumulated over 8 chunks in PSUM, final dot with U.

  Engine mapping per 1024-element supertile: x broadcast via rank-1
  matmul (PE), hat build Abs on ACT + min on DVE (the hat sign is negated
  and absorbed into the negated grid tables so one DVE op suffices),
  interpolation + core contraction on PE in fp16 (1 cycle/row), V/W
  replication on the DMA fabric, K2 product on DVE (fp16 2x mode) with a
  few chunks optionally on GPSIMD.

  Layer-1 sines use ACT's Sin (valid |arg| <= ~3.555) via the "turns"
  reduction: f = w'*(g-0.5) + c'' with c'' host-folded into [-1/4,1/4]
  (sign flips absorbed into layer-2 weight columns), then
  sin(2*pi*f) = sin(4*(w1*g + b1)) exactly.  Layer-2 args are bounded by
  4*(sin(1)*max_row_sum|w2| + max|b2|) < 3.55 (asserted on host).
"""
import os
import sys

for _p in ("/opt/trn_rl_repo", "/root/.axon_site/_ro/trn_rl_repo"):
    if _p not in sys.path:
        sys.path.insert(0, _p)

import numpy as np

import concourse.bass as bass
import concourse.mybir as mybir
import concourse.tile as tile
from concourse import bacc
from concourse.bass_utils import run_bass_kernel_spmd

f32 = mybir.dt.float32
f32r = mybir.dt.float32r
f16 = mybir.dt.float16
AF = mybir.ActivationFunctionType
OP = mybir.AluOpType

N_CORES = 8
B = 65536
B_CORE = B // N_CORES
SUPER = 1024
NSUP = B_CORE // SUPER
NS = SUPER // 512
MID = 512
R = 32
NG = 128          # grid points per net
NCELL = float(NG - 1)
OMEGA = 4.0
TWO_PI = float(2.0 * np.pi)

_CACHE = {}
KLOOP = int(os.environ.get("KLOOP", "0"))       # hardware-loop repeat (timing)
K2POOL = int(os.environ.get("K2POOL", "4"))     # k2 chunks on gpsimd
PSDMA = os.environ.get("PSDMA", "0") == "1"     # DMA outputs straight from PSUM


def _emit(nc, tc, d, out, P):
    """Emit one full kernel pass (const loads + grid eval + batch loop)."""
    const = P["const"]
    sbS = P["sbS"]
    work = P["work"]
    ps_zb = P["ps_zb"]
    ps_it = P["ps_it"]
    ps_t2 = P["ps_t2"]
    ps_o = P["ps_o"]

    # ---------------- constants into SBUF ----------------
    w2sb = [
        [const.tile([128, MID], f16, name=f"w2sb_{n}_{k}") for k in range(4)]
        for n in range(3)
    ]
    w3sb = [
        [const.tile([128, R], f16, name=f"w3sb_{n}_{k}") for k in range(4)]
        for n in range(3)
    ]
    wpsb, c2sb, b2sb4, nb3sb = [], [], [], []
    for n in range(3):
        for k in range(4):
            nc.gpsimd.dma_start(out=w2sb[n][k], in_=d["w2t16"].ap()[n, k])
            nc.gpsimd.dma_start(out=w3sb[n][k], in_=d["w3t16"].ap()[n, k])
        t = const.tile([128, 4], f32, name=f"wpsb_{n}")
        nc.sync.dma_start(out=t, in_=d["wpc"].ap()[n])
        wpsb.append(t)
        t = const.tile([128, 4], f32, name=f"c2sb_{n}")
        nc.sync.dma_start(out=t, in_=d["c2c"].ap()[n])
        c2sb.append(t)
        t = const.tile([128, 4], f32, name=f"b2sb4_{n}")
        nc.sync.dma_start(out=t, in_=d["b2s4"].ap()[n])
        b2sb4.append(t)
        t = const.tile([128, R], f32, name=f"nb3sb_{n}")
        nc.sync.dma_start(out=t, in_=d["nb3bc"].ap()[n])
        nb3sb.append(t)
    gbcsb = const.tile([128, NG], f32, name="gbcsb")
    nc.sync.dma_start(out=gbcsb, in_=d["gbc"].ap())
    npsb = const.tile([128, 1], f32, name="npsb")
    nc.sync.dma_start(out=npsb, in_=d["npvec"].ap())
    one128 = const.tile([1, 128], f32r, name="one128")
    nc.sync.dma_start(out=one128, in_=d["one128"].ap())
    ones32 = const.tile([R, 1], f16, name="ones32")
    nc.sync.dma_start(out=ones32, in_=d["ones32"].ap())
    c3sb = [const.tile([128, R], f16, name=f"c3sb_{c}") for c in range(8)]
    for c in range(8):
        nc.gpsimd.dma_start(out=c3sb[c], in_=d["c316"].ap()[c])

    # ---------------- grid eval: GT[n] = -(net_n(grid) + b3) ----------------
    GT = []
    with tc.tile_pool(name="ps_g", bufs=1, space="PSUM") as ps_g:
        for n in range(3):
            fg = work.tile([128, 4, NG], f32, name="fg", tag="fg", bufs=1)
            for m in range(4):
                nc.vector.tensor_scalar(
                    fg[:, m, :], gbcsb, wpsb[n][:, m : m + 1],
                    c2sb[n][:, m : m + 1], OP.mult, OP.add,
                )
            nc.scalar.activation(fg, fg, AF.Sin, scale=TWO_PI)
            h1g = work.tile([128, 4, NG], f16, name="h1g", tag="h1g", bufs=1)
            nc.scalar.activation(h1g, fg, AF.Sin)
            h2g = work.tile([128, 4, NG], f16, name="h2g", tag="h2g", bufs=1)
            for m in range(4):
                pg = ps_g.tile([128, NG], f32, name="pg", tag="pg")
                for k in range(4):
                    nc.tensor.matmul(
                        pg,
                        lhsT=w2sb[n][k][:, m * 128 : (m + 1) * 128],
                        rhs=h1g[:, k, :],
                        start=(k == 0),
                        stop=(k == 3),
                    )
                tg = work.tile([128, NG], f32, name="tg", tag="tg", bufs=2)
                nc.scalar.activation(
                    tg, pg, AF.Sin, bias=b2sb4[n][:, m : m + 1], scale=OMEGA
                )
                nc.scalar.activation(h2g[:, m, :], tg, AF.Sin)
            pgt = ps_g.tile([128, R], f32, name="pgt", tag="pg")
            for k in range(4):
                nc.tensor.matmul(
                    pgt, lhsT=h2g[:, k, :], rhs=w3sb[n][k],
                    start=(k == 0), stop=(k == 3),
                )
            gt = const.tile([128, R], f16, name=f"GT_{n}")
            nc.vector.tensor_sub(gt, nb3sb[n], pgt)  # -(G + b3)
            GT.append(gt)

    out2d = out.ap().rearrange("(a b) -> a b", a=NSUP)
    xr_ap = d["xr"].ap()

    # ---------------- batch supertile loop ----------------
    for st in range(NSUP):
        xrow = work.tile([1, 3, SUPER], f32r, name="xrow", tag="xrow", bufs=2)
        nc.sync.dma_start(
            out=xrow,
            in_=xr_ap[:, st * SUPER : (st + 1) * SUPER].unsqueeze(0),
        )
        # U/V/W interpolations land in ONE psum tile at base partitions
        # 0/32/64; U stays in PSUM until the final product, V/W get one
        # fused copy to SBUF f16 for the replication DMAs.
        puvw = ps_it.tile([96, NS, 512], f32, name="puvw", tag="it", bufs=1)
        for n in range(3):
            # x broadcast to 128 partitions via rank-1 matmul
            zb = ps_zb.tile([128, NS, 512], f32, name="zb", tag="zb", bufs=1)
            for s2 in range(NS):
                nc.tensor.matmul(
                    zb[:, s2, :],
                    lhsT=one128,
                    rhs=xrow[:, n, s2 * 512 : (s2 + 1) * 512],
                    start=True, stop=True,
                )
            # t1 = |127*x - p|  (ACT), S = min(t1-1, 0) = -hat (DVE)
            t1 = work.tile([128, NS, 512], f16, name="t1", tag="t1", bufs=2)
            nc.scalar.activation(t1, zb, AF.Abs, bias=npsb, scale=NCELL)
            S = sbS.tile([128, NS, 512], f16, name="S", tag=f"S{n}", bufs=2)
            nc.vector.tensor_scalar(S, t1, 1.0, 0.0, OP.subtract, OP.min)
            for s2 in range(NS):
                nc.tensor.matmul(
                    puvw[n * R : (n + 1) * R, s2, :],
                    lhsT=GT[n], rhs=S[:, s2, :], start=True, stop=True,
                )
        vwsb = sbS.tile([64, NS, 512], f16, name="vwsb", tag="vwsb", bufs=2)
        nc.scalar.copy(vwsb, puvw[R : 3 * R, :, :])

        # ---- V/W partition replication DMAs (leading AP dim must step)
        # wrep[p] = W[p % 32]: one DMA, dest partitions enumerated t-major
        wrep = sbS.tile([128, SUPER], f16, name="wrep", tag="wrep", bufs=2)
        nc.sync.dma_start(
            out=wrep[:, :].rearrange("(j t) f -> t j f", j=4),
            in_=vwsb[R : 2 * R, :, :]
            .rearrange("t n f -> t (n f)")
            .unsqueeze(1)
            .broadcast_to([R, 4, SUPER]),
        )
        # vrall[p, c] = V[4c + p//32]: per-chunk broadcast, src [4, 32, S]
        vrall = sbS.tile([128, 8, SUPER], f16, name="vrall", tag="vrall", bufs=2)
        for c in range(8):
            vsrc = (
                vwsb[4 * c : 4 * c + 4, :, :]
                .rearrange("a n f -> a (n f)")
                .unsqueeze(1)
                .broadcast_to([4, R, SUPER])
            )
            nc.sync.dma_start(out=vrall[:, c, :], in_=vsrc)

        # ---- K2 product + core contraction
        t2 = ps_t2.tile([R, NS, 512], f32, name="t2", tag="t2", bufs=1)
        for c in range(8):
            k2 = work.tile([128, SUPER], f16, name="k2", tag="k2", bufs=3)
            eng = nc.gpsimd if c < K2POOL else nc.vector
            eng.tensor_mul(k2, vrall[:, c, :], wrep)
            for s2 in range(NS):
                nc.tensor.matmul(
                    t2[:, s2, :],
                    lhsT=c3sb[c],
                    rhs=k2[:, s2 * 512 : (s2 + 1) * 512],
                    start=(c == 0),
                    stop=(c == 7),
                )
        # ---- final dot with U and reduce over r
        po = ps_o.tile([1, NS, 512], f32, name="po", tag="po", bufs=1)
        for s2 in range(NS):
            m3 = work.tile([R, 512], f16, name="m3", tag="m3", bufs=2)
            nc.vector.tensor_mul(
                m3, t2[:, s2, :], Usb[:, s2 * 512 : (s2 + 1) * 512]
            )
            nc.tensor.matmul(
                po[:, s2, :], lhsT=ones32, rhs=m3, start=True, stop=True
            )
        if PSDMA:
            nc.sync.dma_start(out=out2d[st : st + 1, :], in_=po)
        else:
            orow = work.tile([1, NS, 512], f32, name="orow", tag="orow", bufs=2)
            nc.scalar.copy(orow, po)
            nc.sync.dma_start(out=out2d[st : st + 1, :], in_=orow)


def _build_body(nc, tc, d, out, kloop):
    import contextlib

    with (
        tc.tile_pool(name="const", bufs=1) as const,
        tc.tile_pool(name="sbS", bufs=1) as sbS,
        tc.tile_pool(name="work", bufs=1) as work,
        tc.tile_pool(name="ps_zb", bufs=1, space="PSUM") as ps_zb,
        tc.tile_pool(name="ps_it", bufs=1, space="PSUM") as ps_it,
        tc.tile_pool(name="ps_t2", bufs=1, space="PSUM") as ps_t2,
        tc.tile_pool(name="ps_o", bufs=1, space="PSUM") as ps_o,
    ):
        P = dict(const=const, sbS=sbS, work=work, ps_zb=ps_zb,
                 ps_it=ps_it, ps_t2=ps_t2, ps_o=ps_o)
        loop_cm = (
            tc.For_i(0, kloop, 1) if kloop > 0 else contextlib.nullcontext()
        )
        with loop_cm:
            _emit(nc, tc, d, out, P)


def build_nc(kloop=0):
    nc = bacc.Bacc(
        "TRN2", target_bir_lowering=False, debug=False, num_devices=N_CORES
    )
    d = {}
    specs = (
        ("xr", (3, B_CORE), f32r),
        ("wpc", (3, 128, 4), f32),
        ("c2c", (3, 128, 4), f32),
        ("b2s4", (3, 128, 4), f32),
        ("w2t16", (3, 4, 128, MID), f16),
        ("w3t16", (3, 4, 128, R), f16),
        ("nb3bc", (3, 128, R), f32),
        ("gbc", (128, NG), f32),
        ("npvec", (128, 1), f32),
        ("one128", (1, 128), f32r),
        ("ones32", (R, 1), f16),
        ("c316", (8, 128, R), f16),
    )
    for name, shape, dt in specs:
        d[name] = nc.dram_tensor(name, shape, dt, kind="ExternalInput")
    out = nc.dram_tensor("out", (B_CORE,), f32, kind="ExternalOutput")
    with tile.TileContext(nc) as tc:
        _build_body(nc, tc, d, out, kloop)
    nc.compile()
    return nc


def prep_weights(inputs):
    """Host-side packing of weight-derived device inputs (core-independent)."""
    w = {}
    ww = {k: np.asarray(v, np.float32) for k, v in inputs.items()}
    w2t16 = np.empty((3, 4, 128, MID), np.float16)
    w3t16 = np.empty((3, 4, 128, R), np.float16)
    wpc = np.empty((3, 128, 4), np.float32)
    c2c = np.empty((3, 128, 4), np.float32)
    b2s4 = np.empty((3, 128, 4), np.float32)
    nb3bc = np.empty((3, 128, R), np.float32)
    for n, pfx in enumerate(("U", "V", "W")):
        w1 = ww[pfx + "w1"][:, 0]
        b1 = ww[pfx + "b1"]
        w2 = ww[pfx + "w2"]
        b2 = ww[pfx + "b2"]
        w3 = ww[pfx + "w3"]
        b3 = ww[pfx + "b3"]
        # layer-2 arg domain check (ACT sin valid |arg| <= ~3.555)
        bound = OMEGA * (
            np.sin(1.0) * np.abs(w2).sum(axis=1).max() + np.abs(b2).max()
        )
        assert bound < 3.55, f"layer-2 sin arg bound {bound} exceeds ACT domain"
        # layer-1 turns: f = w'*(g-0.5) + c'' ; sign flips into w2 columns
        wp = np.float64(2.0 / np.pi) * w1.astype(np.float64)
        c0 = np.float64(2.0 / np.pi) * b1.astype(np.float64) + 0.5 * wp
        c1 = c0 - np.round(c0)
        flip = np.abs(c1) > 0.25
        c2f = np.where(flip, c1 - 0.5 * np.sign(c1), c1)
        F = np.where(flip, -1.0, 1.0)
        w2_eff = (w2.astype(np.float64) * F[None, :]).astype(np.float32)
        w2t16[n] = w2_eff.T.reshape(4, 128, MID).astype(np.float16)
        w3t16[n] = w3.T.reshape(4, 128, R).astype(np.float16)
        wpc[n] = wp.astype(np.float32).reshape(4, 128).T
        c2c[n] = c2f.astype(np.float32).reshape(4, 128).T
        b2s4[n] = (OMEGA * b2).reshape(4, 128).T
        nb3bc[n] = np.broadcast_to(-b3[None, :], (128, R))
    w["w2t16"], w["w3t16"] = w2t16, w3t16
    w["wpc"], w["c2c"], w["b2s4"], w["nb3bc"] = wpc, c2c, b2s4, nb3bc
    grid = np.arange(NG, dtype=np.float32) / np.float32(NCELL) - 0.5
    w["gbc"] = np.broadcast_to(grid[None, :], (128, NG)).copy()
    w["npvec"] = -np.arange(128, dtype=np.float32).reshape(128, 1)
    w["one128"] = np.ones((1, 128), np.float32)
    w["ones32"] = np.ones((R, 1), np.float16)
    c316 = np.empty((8, 128, R), np.float16)
    q = np.arange(128)
    C = ww["core"].reshape(R, R, R)
    for c in range(8):
        s = 4 * c + q // 32
        c316[c] = C[:, s, q % 32].T
    w["c316"] = c316
    return w


def make_in_maps(inputs):
    w = prep_weights(inputs)
    x = np.asarray(inputs["train_ind_batch"], np.float32)
    in_maps = []
    for c in range(N_CORES):
        sl = x[c * B_CORE : (c + 1) * B_CORE]
        m = dict(w)
        m["xr"] = np.ascontiguousarray(sl.T)
        in_maps.append(m)
    return in_maps


def get_nc():
    if "nc" not in _CACHE:
        _CACHE["nc"] = build_nc(KLOOP)
    return _CACHE["nc"]


def kernel(**inputs) -> np.ndarray:
    nc = get_nc()
    in_maps = make_in_maps(inputs)
    res = run_bass_kernel_spmd(nc, in_maps, core_ids=list(range(N_CORES)))
    return np.concatenate(
        [res.results[c]["out"] for c in range(N_CORES)]
    ).astype(np.float32)


if __name__ == "__main__":
    rng = np.random.default_rng(0)
    demo = {"train_ind_batch": rng.uniform(0, 1, (B, 3)).astype(np.float32)}
    for pfx in ("U", "V", "W"):
        demo[pfx + "w1"] = rng.uniform(-1, 1, (MID, 1)).astype(np.float32)
        demo[pfx + "b1"] = rng.uniform(-1, 1, MID).astype(np.float32)
        demo[pfx + "w2"] = rng.uniform(-1 / MID, 1 / MID, (MID, MID)).astype(
            np.float32
        )
        demo[pfx + "b2"] = rng.uniform(
            -1 / np.sqrt(MID), 1 / np.sqrt(MID), MID
        ).astype(np.float32)
        demo[pfx + "w3"] = rng.uniform(
            -1 / np.sqrt(MID), 1 / np.sqrt(MID), (R, MID)
        ).astype(np.float32)
        demo[pfx + "b3"] = rng.uniform(
            -1 / np.sqrt(MID), 1 / np.sqrt(MID), R
        ).astype(np.float32)
    demo["core"] = rng.standard_normal(R * R * R).astype(np.float32)
    out = kernel(**demo)
    print("out", out.shape, out[:4])
